# revision 1
# baseline (speedup 1.0000x reference)
"""DeepNCM Trainium2 kernel: prototype scatter-mean update + negative squared
L2 distances, data-parallel over embedding rows across 8 NeuronCores.

Contract: kernel(**inputs) takes the FULL unsharded inputs
(embeddings [65536,512] f32, prototypes [1000,512] f32, counter [1000] f32,
y_true [65536] int64) and returns the FULL output [65536,1000] f32.

Per-core plan (N_loc = 8192 rows):
  Phase 1: segment sums via one-hot matmul (lhsT=emb tile, rhs=onehot tile)
           accumulated in PSUM over 64 k-tiles; counts via DVE accumulation
           of the one-hot tiles + a ones-vector matmul reduction; e_sq via
           ScalarE Square with free-dim accumulation.
  AllReduce of [sums ; counts] (513x1000 f32) across the 8 cores.
  Prototype update (scatter_mean + running mean + where) computed per-class
  on-device, replicated on every core.
  Phase 2: cross = emb @ (2*protos)^T via PE (emb transposed on the fly with
           PE transpose-mode), epilogue out = 2*cross - e_sq - p_sq fused
           into ScalarE (per-partition bias) + VectorE (p_sq broadcast row).

Matmul operands are bf16 (accumulation in fp32 PSUM); everything scale-
sensitive (e_sq, prototype math, epilogue) stays fp32.
"""

import os
import sys
from contextlib import ExitStack

for _p in ("/opt/trn_rl_repo", "/root/.axon_site/_ro/trn_rl_repo"):
    if os.path.isdir(_p):
        if _p not in sys.path:
            sys.path.insert(0, _p)
        break

import numpy as np

import concourse.bass as bass
import concourse.mybir as mybir
import concourse.tile as tile
from concourse.masks import make_identity
from concourse.bass_utils import run_bass_kernel_spmd

N, D, C = 65536, 512, 1000
W = 8                      # cores
NL = N // W                # rows per core
P = 128
KT = NL // P               # 64 row tiles per core
DC = D // P                # 4 contraction chunks
CH = ((0, 512), (512, 1000))   # free-dim halves of the class axis
F32 = mybir.dt.float32
BF16 = mybir.dt.bfloat16
ALU = mybir.AluOpType
ACTF = mybir.ActivationFunctionType

# Toggled by test.py for profiling runs.
PROFILE = False
TRACE_KWARGS = {}
LAST_RESULT = [None]

_built = [None]


def _split_waits(nc, cap=1):
    """Walrus in this container rejects >1 sync-wait per instruction.
    Move excess waits onto preceding same-engine NOPs (in-order engines,
    so semantics are preserved)."""
    n_new = 0
    for fn in nc.m.functions:
        for bb in fn.blocks:
            new_list = []
            for ins in bb.instructions:
                si = getattr(ins, "sync_info", None)
                if si is not None and si.on_wait and len(si.on_wait) > cap:
                    waits = list(si.on_wait)
                    keep, rest = waits[:cap], waits[cap:]
                    for i in range(0, len(rest), cap):
                        nop = mybir.InstNoOp(
                            name=f"I-waitsplit-{n_new}", ins=[], outs=[]
                        )
                        n_new += 1
                        nop.engine = ins.engine
                        nop.sync_info = mybir.SyncInfo(
                            on_wait=rest[i : i + cap], on_update=[]
                        )
                        new_list.append(nop)
                    si.on_wait = keep
                new_list.append(ins)
            bb.instructions = new_list
    return n_new


def _build(unroll=1):
    nc = bass.Bass()
    emb_ext = nc.declare_dram_parameter("emb", [NL, D], F32, isOutput=False)
    yf_ext = nc.declare_dram_parameter("yf", [P, KT], F32, isOutput=False)
    counter_ext = nc.declare_dram_parameter("counter", [C], F32, isOutput=False)
    p0_ext = nc.declare_dram_parameter("p0", [C, D], F32, isOutput=False)
    out_ext = nc.declare_dram_parameter("out", [NL, C], F32, isOutput=True)

    with tile.TileContext(nc) as tc, ExitStack() as es:
        cpool = es.enter_context(tc.tile_pool(name="const", bufs=1))
        bpool = es.enter_context(tc.tile_pool(name="bigs", bufs=1))
        rpool = es.enter_context(tc.tile_pool(name="rows", bufs=1))
        in_pool = es.enter_context(tc.tile_pool(name="inp", bufs=4))
        oh_pool = es.enter_context(tc.tile_pool(name="oh", bufs=3))
        sq_pool = es.enter_context(tc.tile_pool(name="sq", bufs=2))
        etb_pool = es.enter_context(tc.tile_pool(name="etb", bufs=3))
        out_pool = es.enter_context(tc.tile_pool(name="outp", bufs=8))
        tmpb_pool = es.enter_context(tc.tile_pool(name="tmpb", bufs=2))
        dram = es.enter_context(tc.tile_pool(name="dram", bufs=1, space="DRAM"))

        # ---- constants ----
        ident = cpool.tile([P, P], F32, name="ident")
        make_identity(nc, ident[:])
        iota = cpool.tile([P, C], F32, name="iota")
        nc.gpsimd.iota(
            iota[:], pattern=[[1, C]], base=0, channel_multiplier=0,
            allow_small_or_imprecise_dtypes=True,
        )
        ones_col = cpool.tile([P, 1], BF16, name="onesc")
        nc.vector.memset(ones_col[:], 1.0)
        ones_row = cpool.tile([1, P], BF16, name="onesr")
        nc.vector.memset(ones_row[:], 1.0)

        y_sb = cpool.tile([P, KT], F32, name="y")
        nc.sync.dma_start(y_sb[:], yf_ext[:])
        e_sq = cpool.tile([P, KT], F32, name="esq")
        counts_acc = cpool.tile([P, C], BF16, name="cacc")
        nc.vector.memset(counts_acc[:], 0.0)

        sums_sb = bpool.tile([P, DC * C], BF16, name="sums")
        p0T = bpool.tile([P, DC * C], F32, name="p0T")  # later holds protosT
        A_b = bpool.tile([P, C], F32, tag="Abt", name="Ab")
        B_b = bpool.tile([P, C], F32, tag="Bbt", name="Bb")
        embT_full = bpool.tile([P, KT * D], BF16, name="embTf")

        for it_ in range(unroll):

            with tc.tile_pool(name=f"ps_sums{it_}", bufs=1, space="PSUM") as ps_sums:
                s_ps = [
                    [ps_sums.tile([P, c1 - c0], F32, tag=f"s{dc}_{ci}",
                                  name=f"s{dc}_{ci}")
                     for ci, (c0, c1) in enumerate(CH)]
                    for dc in range(DC)
                ]
                for kt in range(KT):
                    et = in_pool.tile([P, D], F32, tag="et", name="et")
                    nc.sync.dma_start(et[:], emb_ext[kt * P : (kt + 1) * P, :])
                    scr = sq_pool.tile([P, D], F32, tag="scr", name="scr")
                    nc.scalar.activation(
                        scr[:], et[:], ACTF.Square,
                        accum_out=e_sq[:, kt : kt + 1],
                    )
                    etb = etb_pool.tile([P, D], BF16, tag="etb", name="etb")
                    nc.gpsimd.tensor_copy(out=etb[:], in_=et[:])
                    oh = oh_pool.tile([P, C], BF16, tag="oh", name="oh")
                    nc.vector.tensor_scalar(
                        oh[:], iota[:], y_sb[:, kt : kt + 1], None, ALU.is_equal
                    )
                    nc.vector.tensor_tensor(
                        out=counts_acc[:], in0=counts_acc[:], in1=oh[:], op=ALU.add
                    )
                    for dc in range(DC):
                        lhs = etb[:, dc * P : (dc + 1) * P]
                        for ci, (c0, c1) in enumerate(CH):
                            nc.tensor.matmul(
                                s_ps[dc][ci][:], lhs, oh[:, c0:c1],
                                start=(kt == 0), stop=(kt == KT - 1),
                            )

                # negate e_sq once (used as ScalarE bias in phase 2)
                nc.vector.tensor_scalar(e_sq[:], e_sq[:], -1.0, None, ALU.mult)

                # sums psum -> sbuf (split between ScalarE / VectorE)
                for dc in range(DC):
                    for ci, (c0, c1) in enumerate(CH):
                        dst = sums_sb[:, dc * C + c0 : dc * C + c1]
                        if (dc + ci) % 2 == 0:
                            nc.scalar.copy(dst, s_ps[dc][ci][:])
                        else:
                            nc.vector.tensor_copy(out=dst, in_=s_ps[dc][ci][:])

            # ---- mid-kernel psum work: counts reduce, p0 transpose, coeffs ----
            with tc.tile_pool(name=f"ps_mid{it_}", bufs=1, space="PSUM") as ps_mid:
                # counts: reduce over partitions with ones-vector matmul
                counts_row = rpool.tile([1, C], F32, name="counts")
                for ci, (c0, c1) in enumerate(CH):
                    cp = ps_mid.tile([1, c1 - c0], F32, tag=f"r{ci}",
                                     name=f"cnt{ci}")
                    nc.tensor.matmul(
                        cp[:], ones_col[:], counts_acc[:, c0:c1],
                        start=True, stop=True,
                    )
                    nc.scalar.copy(counts_row[:, c0:c1], cp[:])

                # ---- all-reduce #1: counts only (tiny, finishes fast) ----
                cc1_in = dram.tile([1, C], F32, tag=f"c1i{it_}", name="c1i")
                cc1_out = dram.tile([1, C], F32, tag=f"c1o{it_}", name="c1o",
                                    addr_space="Shared")
                nc.sync.dma_start(cc1_in[:], counts_row[:])
                nc.gpsimd.collective_compute(
                    "AllReduce", ALU.add,
                    replica_groups=[list(range(W))],
                    ins=[cc1_in.opt()], outs=[cc1_out.opt()],
                )
                nc.sync.dma_start(counts_row[:], cc1_out[:])

                # ---- all-reduce #2: sums (big; overlapped with coeff math,
                # p0 transposes and the phase-2 transpose pre-staging) ----
                cc_in = dram.tile([DC * P, C], BF16, tag=f"ccin{it_}", name="ccin")
                cc_out = dram.tile([DC * P, C], BF16, tag=f"ccout{it_}",
                                   name="ccout", addr_space="Shared")
                for dc in range(DC):
                    nc.sync.dma_start(
                        cc_in[dc * P : (dc + 1) * P, :],
                        sums_sb[:, dc * C : (dc + 1) * C],
                    )
                nc.gpsimd.collective_compute(
                    "AllReduce", ALU.add,
                    replica_groups=[list(range(W))],
                    ins=[cc_in.opt()], outs=[cc_out.opt()],
                )
                for dc in range(DC):
                    nc.sync.dma_start(
                        sums_sb[:, dc * C : (dc + 1) * C],
                        cc_out[dc * P : (dc + 1) * P, :],
                    )

                # ---- p0 load + transpose (overlaps the collectives) ----
                for ct in range(8):
                    ncp = min(P, C - ct * P)
                    pt = in_pool.tile([P, D], F32, tag="et", name="p0t")
                    nc.sync.dma_start(
                        pt[0:ncp, :], p0_ext[ct * P : ct * P + ncp, :]
                    )
                    tr = ps_mid.tile([P, DC * P], F32, tag="tr", bufs=3,
                                     name="tr")
                    for dc in range(DC):
                        nc.tensor.matmul(
                            tr[:, dc * P : dc * P + ncp],
                            pt[0:ncp, dc * P : (dc + 1) * P],
                            ident[0:ncp, 0:ncp],
                            is_transpose=True,
                            start=(dc == 0), stop=(dc == DC - 1),
                        )
                    # strided single copy: psum block dc -> p0T chunk dc
                    dst = p0T.rearrange("p (dc c) -> p dc c", dc=DC)[
                        :, :, ct * P : ct * P + ncp]
                    srcv = tr.rearrange("p (dc q) -> p dc q", dc=DC)[:, :, 0:ncp]
                    if ct % 2 == 0:
                        nc.scalar.copy(dst, srcv)
                    else:
                        nc.vector.tensor_copy(out=dst, in_=srcv)

                # ---- pre-stage ALL phase-2 emb transposes (hidden under CC) ----
                for nt in range(KT):
                    et = in_pool.tile([P, D], F32, tag="et", name="et")
                    nc.sync.dma_start(et[:], emb_ext[nt * P : (nt + 1) * P, :])
                    tr = ps_mid.tile([P, DC * P], F32, tag="tr", bufs=3,
                                     name="tr")
                    for dc in range(DC):
                        nc.tensor.matmul(
                            tr[:, dc * P : (dc + 1) * P],
                            et[:, dc * P : (dc + 1) * P],
                            ident[:],
                            is_transpose=True,
                            start=(dc == 0), stop=(dc == DC - 1),
                        )
                    dst = embT_full[:, nt * D : (nt + 1) * D]
                    if nt % 2 == 0:
                        nc.scalar.copy(dst, tr[:])
                    else:
                        nc.vector.tensor_copy(out=dst, in_=tr[:])

                counter_row = rpool.tile([1, C], F32, name="ctr")
                nc.sync.dma_start(counter_row[:], counter_ext[None, :])

                # ---- per-class prototype coefficients (needs counts AR only) ----
                # protos = where(counts>0,
                #                (counter*p0 + sums/max(counts,1)) / (counter+1),
                #                p0)
                #        = A*p0 + B*sums;  we build 2A and 2B so the matmul rhs
                # protos2 = 2*protosT folds the cross-term factor of 2.
                rep = rpool.tile([1, C], F32, name="rep")
                nc.vector.tensor_scalar(rep[:], counts_row[:], 0.0, None, ALU.is_gt)
                tmp1 = rpool.tile([1, C], F32, tag="t1", name="t1")
                tmp2 = rpool.tile([1, C], F32, tag="t2", name="t2")
                A_row = rpool.tile([1, C], BF16, name="A")
                B_row = rpool.tile([1, C], BF16, name="B")
                # rm = 1/max(counts,1)
                nc.vector.tensor_scalar(tmp1[:], counts_row[:], 1.0, None, ALU.max)
                nc.vector.reciprocal(tmp1[:], tmp1[:])
                # rt = 1/(counter+1)
                nc.vector.tensor_scalar(tmp2[:], counter_row[:], 1.0, None, ALU.add)
                nc.vector.reciprocal(tmp2[:], tmp2[:])
                # 2B = 2 * rep * rm * rt
                nc.vector.tensor_tensor(out=B_row[:], in0=tmp1[:], in1=tmp2[:],
                                        op=ALU.mult)
                nc.vector.tensor_tensor(out=B_row[:], in0=B_row[:], in1=rep[:],
                                        op=ALU.mult)
                nc.vector.tensor_scalar(B_row[:], B_row[:], 2.0, None, ALU.mult)
                # 2A = 2 * (1 + rep * (counter*rt - 1))
                nc.vector.tensor_tensor(out=A_row[:], in0=counter_row[:],
                                        in1=tmp2[:], op=ALU.mult)
                nc.vector.tensor_scalar(A_row[:], A_row[:], 1.0, None, ALU.subtract)
                nc.vector.tensor_tensor(out=A_row[:], in0=A_row[:], in1=rep[:],
                                        op=ALU.mult)
                nc.vector.tensor_scalar(A_row[:], A_row[:], 1.0, None, ALU.add)
                nc.vector.tensor_scalar(A_row[:], A_row[:], 2.0, None, ALU.mult)

                # broadcast 2A,2B down partitions via ones outer-product
                for row, dst_b in ((A_row, A_b), (B_row, B_b)):
                    for ci, (c0, c1) in enumerate(CH):
                        ob = ps_mid.tile([P, c1 - c0], F32, tag="ob", bufs=2,
                                         name="ob")
                        nc.tensor.matmul(
                            ob[:], ones_row[:], row[:, c0:c1],
                            start=True, stop=True,
                        )
                        nc.scalar.copy(dst_b[:, c0:c1], ob[:])

                # p0T *= 2A (can run during the sums all-reduce)
                for dc in range(DC):
                    sl = slice(dc * C, (dc + 1) * C)
                    nc.vector.tensor_tensor(out=p0T[:, sl], in0=p0T[:, sl],
                                            in1=A_b[:], op=ALU.mult)

                # protos2 = 2A*p0T + 2B*sums  (bf16, the phase-2 matmul rhs)
                # interleaved with p_sq = 0.25 * sum_d protos2^2 per chunk
                protos2 = bpool.tile([P, DC * C], BF16, tag="pr2", name="pr2")
                psq_ps = [ps_mid.tile([1, c1 - c0], F32, tag=f"r{ci}",
                                      name=f"psq{ci}")
                          for ci, (c0, c1) in enumerate(CH)]
                for dc in range(DC):
                    sl = slice(dc * C, (dc + 1) * C)
                    tb = tmpb_pool.tile([P, C], F32, tag="tb", name="tb")
                    nc.vector.tensor_tensor(out=tb[:], in0=sums_sb[:, sl],
                                            in1=B_b[:], op=ALU.mult)
                    nc.vector.tensor_tensor(out=protos2[:, sl], in0=p0T[:, sl],
                                            in1=tb[:], op=ALU.add)
                    tbq = tmpb_pool.tile([P, C], BF16, tag="tbq", name="tbq")
                    nc.vector.tensor_tensor(out=tbq[:], in0=protos2[:, sl],
                                            in1=protos2[:, sl], op=ALU.mult)
                    for ci, (c0, c1) in enumerate(CH):
                        nc.tensor.matmul(
                            psq_ps[ci][:], ones_col[:], tbq[:, c0:c1],
                            start=(dc == 0), stop=(dc == DC - 1),
                        )
                p_sq_row = rpool.tile([1, C], BF16, tag="psqr", name="psqr")
                for ci, (c0, c1) in enumerate(CH):
                    nc.scalar.copy(p_sq_row[:, c0:c1], psq_ps[ci][:])
                p_sq_b = bpool.tile([P, C], F32, tag="Abt", name="psqb")
                for ci, (c0, c1) in enumerate(CH):
                    ob = ps_mid.tile([P, c1 - c0], F32, tag="ob", bufs=2, name="ob")
                    nc.tensor.matmul(
                        ob[:], ones_row[:], p_sq_row[:, c0:c1],
                        start=True, stop=True,
                    )
                    nc.scalar.mul(p_sq_b[:, c0:c1], ob[:], 0.25)

            # ---- phase 2: out = 2*emb@protosT' - e_sq - p_sq ----
            with tc.tile_pool(name=f"ps_cr{it_}", bufs=4, space="PSUM") as ps_cr:
                for nt in range(KT):
                    ot = out_pool.tile([P, C], F32, tag="ot", name="ot")
                    for ci, (c0, c1) in enumerate(CH):
                        cr = ps_cr.tile([P, c1 - c0], F32, tag=f"cr{ci}",
                                        name=f"cr{ci}")
                        for dc in range(DC):
                            nc.tensor.matmul(
                                cr[:],
                                embT_full[:, nt * D + dc * P : nt * D + (dc + 1) * P],
                                protos2[:, dc * C + c0 : dc * C + c1],
                                start=(dc == 0), stop=(dc == DC - 1),
                            )
                        nc.scalar.activation(
                            ot[:, c0:c1], cr[:], ACTF.Identity,
                            bias=e_sq[:, nt : nt + 1], scale=1.0,
                        )
                    nc.vector.tensor_tensor(
                        out=ot[:], in0=ot[:], in1=p_sq_b[:], op=ALU.subtract
                    )
                    nc.sync.dma_start(out_ext[nt * P : (nt + 1) * P, :], ot[:])

    _split_waits(nc)
    return nc


def kernel(embeddings, prototypes, counter, y_true):
    embeddings = np.ascontiguousarray(np.asarray(embeddings, dtype=np.float32))
    prototypes = np.ascontiguousarray(np.asarray(prototypes, dtype=np.float32))
    counter_f = np.ascontiguousarray(np.asarray(counter, dtype=np.float32))
    y = np.asarray(y_true)

    if _built[0] is None:
        _built[0] = _build()
    nc = _built[0]

    in_maps = []
    for i in range(W):
        sl = slice(i * NL, (i + 1) * NL)
        y_loc = y[sl].astype(np.float32)
        # partition-major labels: yf[p, t] = y_loc[t*128 + p]
        yf = np.ascontiguousarray(y_loc.reshape(KT, P).T)
        in_maps.append(
            {
                "emb": embeddings[sl],
                "yf": yf,
                "counter": counter_f,
                "p0": prototypes,
            }
        )

    res = run_bass_kernel_spmd(
        nc, in_maps, list(range(W)), trace=PROFILE, **TRACE_KWARGS
    )
    LAST_RESULT[0] = res
    out = np.concatenate([res.results[i]["out"] for i in range(W)], axis=0)
    return out.astype(np.float32, copy=False)



# revision 37
# speedup vs baseline: 2.0814x; 2.0814x over previous
"""DeepNCM Trainium2 kernel v3: fp8 DoubleRow one-hot segment sums +
fp8 DoubleRow distance GEMM, data-parallel over embedding rows across 8 cores.

Contract: kernel(**inputs) takes the FULL unsharded inputs
(embeddings [65536,512] f32, prototypes [1000,512] f32, counter [1000] f32,
y_true [65536] int64) and returns the FULL output [65536,1000] f32.

Per-core plan (NL = 8192 rows):
  Host precomputes counts = bincount(y) and folds the running-mean update
  into per-class coefficients: protos2 = A2*p0 + B2*sums (factor 2 folded).
  The host also ships A2*p0^T and broadcast B2 so the device applies them
  with two elementwise ops.
  Phase 1 (per pair of 128-row tiles): DMA f32 emb; quantize to fp8;
  ACT computes e_sq (Square+accum from f32); DVE/Pool build fp8 one-hot
  tiles; PE accumulates sumsT[d,c] += emb^T @ onehot with DoubleRow fp8
  matmuls (two row-tiles per instruction). PSUM sums -> bf16 -> DRAM.
  ReduceScatter gives each core a 64-row D-slice of the reduced sumsT;
  it computes its protos2T slice + a -p_sq/4 partial row, quantizes to
  fp8, and an AllGather replicates [8*(64+1), 1000] fp8 — already in the
  [D, C] layout phase 2 needs. p_sq partials are summed with a tiny
  ones-matmul into the K=1 fold rows.
  Phase 2: cross via fp8 DoubleRow matmuls + a K=1 DoubleRow instruction
  folding -p_sq into PSUM; ACT/DVE epilogue adds -e_sq (per-partition
  bias) and writes fp16; host upcasts. Emb transposes (PE, fp8, stride-2
  PSUM) and their SBUF copies run inside the collective window.
"""

import os
import sys
from contextlib import ExitStack

for _p in ("/opt/trn_rl_repo", "/root/.axon_site/_ro/trn_rl_repo"):
    if os.path.isdir(_p):
        if _p not in sys.path:
            sys.path.insert(0, _p)
        break

import numpy as np

import concourse.bass as bass
import concourse.mybir as mybir
import concourse.tile as tile
from concourse.masks import make_identity
from concourse.bass_utils import run_bass_kernel_spmd

N, D, C = 65536, 512, 1000
W = 8                      # cores
NL = N // W                # rows per core
P = 128
KT = NL // P               # 64 row tiles per core
DC = D // P                # 4 D chunks of 128
DS = D // W                # 64-row D-slice per core after ReduceScatter
AGB = DS + 1               # AllGather block: 64 protos2T rows + 1 psq row
CH = ((0, 512), (512, 1000))   # free-dim halves of the class axis
F32 = mybir.dt.float32
F16 = mybir.dt.float16
BF16 = mybir.dt.bfloat16
FP8 = mybir.dt.float8e4
ALU = mybir.AluOpType
ACTF = mybir.ActivationFunctionType
DR = mybir.MatmulPerfMode.DoubleRow

# Toggled by test.py for profiling runs.
PROFILE = False
TRACE_KWARGS = {}
LAST_RESULT = [None]

_built = [None]


def _split_waits(nc, cap=1):
    """Walrus in this container rejects >1 sync-wait per instruction.
    Move excess waits onto preceding same-engine NOPs (in-order engines,
    so semantics are preserved)."""
    n_new = 0
    for fn in nc.m.functions:
        for bb in fn.blocks:
            new_list = []
            for ins in bb.instructions:
                si = getattr(ins, "sync_info", None)
                if si is not None and si.on_wait and len(si.on_wait) > cap:
                    waits = list(si.on_wait)
                    keep, rest = waits[:cap], waits[cap:]
                    for i in range(0, len(rest), cap):
                        nop = mybir.InstNoOp(
                            name=f"I-waitsplit-{n_new}", ins=[], outs=[]
                        )
                        n_new += 1
                        nop.engine = ins.engine
                        nop.sync_info = mybir.SyncInfo(
                            on_wait=rest[i : i + cap], on_update=[]
                        )
                        new_list.append(nop)
                    si.on_wait = keep
                new_list.append(ins)
            bb.instructions = new_list
    return n_new


def _build():
    nc = bass.Bass()
    emb_ext = nc.declare_dram_parameter("emb", [NL, D], F32, isOutput=False)
    yf_ext = nc.declare_dram_parameter("yf", [P, KT], F32, isOutput=False)
    ap0_ext = nc.declare_dram_parameter("ap0", [DS, C], F32, isOutput=False)
    bb_ext = nc.declare_dram_parameter("bb", [DS, C], F32, isOutput=False)
    out_ext = nc.declare_dram_parameter("out", [NL, C], F16, isOutput=True)

    with tile.TileContext(nc) as tc, ExitStack() as es:
        cpool = es.enter_context(tc.tile_pool(name="const", bufs=1))
        bpool = es.enter_context(tc.tile_pool(name="bigs", bufs=1))
        in_pool = es.enter_context(tc.tile_pool(name="inp", bufs=6))
        oh_pool = es.enter_context(tc.tile_pool(name="oh", bufs=4))
        sq_pool = es.enter_context(tc.tile_pool(name="sq", bufs=2))
        out_pool = es.enter_context(tc.tile_pool(name="outp", bufs=6))
        dram = es.enter_context(tc.tile_pool(name="dram", bufs=1, space="DRAM"))

        # ---- constants ----
        ident_8 = cpool.tile([P, P], FP8, name="ident8")
        make_identity(nc, ident_8[:])
        fold_ones = cpool.tile([1, 2 * P], FP8, name="fones")
        nc.vector.memset(fold_ones[:], 1.0)
        ones8 = cpool.tile([P, 1], FP8, name="ones8")
        nc.vector.memset(ones8[:], 1.0)
        iota = cpool.tile([P, C], F32, name="iota")
        nc.gpsimd.iota(
            iota[:], pattern=[[1, C]], base=0, channel_multiplier=0,
            allow_small_or_imprecise_dtypes=True,
        )
        y_sb = cpool.tile([P, KT], F32, name="y")
        nc.sync.dma_start(y_sb[:], yf_ext[:])
        ap0_sb = cpool.tile([P, C], F32, name="ap0")
        nc.sync.dma_start(ap0_sb[0:DS, :], ap0_ext[:])
        bb_sb = cpool.tile([P, C], F32, name="bb")
        nc.sync.dma_start(bb_sb[0:DS, :], bb_ext[:])

        esq_neg = cpool.tile([P, KT], F32, name="esqn")
        e8 = bpool.tile([P, KT * D], FP8, name="e8")
        embT8 = bpool.tile([P, KT * D], FP8, name="embT8")
        protosT8 = bpool.tile([P, DC * C], FP8, name="protosT8")
        fold_rhs = bpool.tile([1, 2 * C], FP8, name="foldr")

        # ================= phase 1: one-hot segment sums =================
        with tc.tile_pool(name="ps_sums", bufs=1, space="PSUM") as ps_sums:
            s_ps = [
                [ps_sums.tile([P, c1 - c0], F32, tag=f"s{dc}_{ci}",
                              name=f"s{dc}_{ci}")
                 for ci, (c0, c1) in enumerate(CH)]
                for dc in range(DC)
            ]
            for kp in range(KT // 2):
                t0 = kp * 2
                et = in_pool.tile([P, 2 * D], F32, tag="et", name="et")
                src = emb_ext[t0 * P : (t0 + 2) * P, :].rearrange(
                    "(i p) d -> p i d", i=2
                )
                ld_eng = nc.sync if kp % 4 != 1 else nc.gpsimd
                ld_eng.dma_start(et.rearrange("p (i d) -> p i d", i=2), src)
                # fp8 quantization (feeds the sums matmuls AND phase 2)
                dst = e8[:, t0 * D : (t0 + 2) * D]
                if kp % 2 == 0:
                    nc.gpsimd.tensor_copy(out=dst, in_=et[:])
                else:
                    nc.vector.tensor_copy(out=dst, in_=et[:])
                # e_sq from f32 (exact); negate later
                for i in range(2):
                    scr = sq_pool.tile([P, D], BF16, tag="scr", name="scr")
                    nc.scalar.activation(
                        scr[:], et[:, i * D : (i + 1) * D], ACTF.Square,
                        accum_out=esq_neg[:, t0 + i : t0 + i + 1],
                    )
                # one-hot pair tile [128, 2, C] fp8
                oh = oh_pool.tile([P, 2 * C], FP8, tag="oh", name="oh")
                for i in range(2):
                    oh_eng = nc.vector if (kp + i) % 3 != 2 else nc.gpsimd
                    oh_eng.tensor_scalar(
                        oh[:, i * C : (i + 1) * C], iota[:],
                        y_sb[:, t0 + i : t0 + i + 1], None, ALU.is_equal,
                    )
                ohv = oh.rearrange("p (pl c) -> p pl c", pl=2)
                e8v = e8.rearrange("p (nt dc m) -> p nt dc m", nt=KT, dc=DC)
                for dc in range(DC):
                    lhs = e8v[:, t0 : t0 + 2, dc, :]  # [P, 2, 128]
                    for ci, (c0, c1) in enumerate(CH):
                        nc.tensor.matmul(
                            s_ps[dc][ci][:],
                            lhs,
                            ohv[:, :, c0:c1],
                            start=(kp == 0), stop=(kp == KT // 2 - 1),
                            perf_mode=DR,
                        )
            # sums psum -> sbuf bf16 (D-major [512, 1000])
            sums_sb = cpool.tile([P, DC * C], BF16, name="sumssb")
            for dc in range(DC):
                for ci, (c0, c1) in enumerate(CH):
                    dsts = sums_sb[:, dc * C + c0 : dc * C + c1]
                    if (dc + ci) % 2 == 0:
                        nc.scalar.copy(dsts, s_ps[dc][ci][:])
                    else:
                        nc.vector.tensor_copy(out=dsts, in_=s_ps[dc][ci][:])

        sums_d = dram.tile([D, C], BF16, name="sumsd")
        for dc in range(DC):
            nc.sync.dma_start(
                sums_d[dc * P : (dc + 1) * P, :],
                sums_sb[:, dc * C : (dc + 1) * C],
            )

        # negate e_sq once (used as ScalarE bias in phase 2)
        nc.vector.tensor_scalar(esq_neg[:], esq_neg[:], -1.0, None, ALU.mult)

        # ---- ReduceScatter: core i owns D rows [64i, 64i+64) ----
        rs_out = dram.tile([DS, C], BF16, name="rsout")
        nc.gpsimd.collective_compute(
            "ReduceScatter", ALU.add,
            replica_groups=[list(range(W))],
            ins=[sums_d.opt()], outs=[rs_out.opt()],
        )
        sums_rs = cpool.tile([P, C], BF16, name="sumsrs")
        nc.sync.dma_start(sums_rs[0:DS, :], rs_out[:])

        # ---- protos2T slice + psq partial, quantize, AllGather ----
        pr2 = cpool.tile([P, C], FP8, name="pr2")
        t2 = cpool.tile([P, C], F32, name="t2")
        nc.vector.tensor_tensor(out=t2[0:DS, :], in0=sums_rs[0:DS, :],
                                in1=bb_sb[0:DS, :], op=ALU.mult)
        nc.vector.tensor_tensor(out=pr2[0:DS, :], in0=t2[0:DS, :],
                                in1=ap0_sb[0:DS, :], op=ALU.add)
        sq8 = cpool.tile([P, C], FP8, name="sq8")
        nc.vector.tensor_tensor(out=sq8[0:DS, :], in0=pr2[0:DS, :],
                                in1=pr2[0:DS, :], op=ALU.mult)

        ag_in = dram.tile([AGB, C], FP8, name="agin")
        ag_out = dram.tile([W * AGB, C], FP8, name="agout",
                           addr_space="Shared")

        def _psq_ag():
            psq8 = cpool.tile([1, C], FP8, name="psq8")
            with tc.tile_pool(name="ps_pq", bufs=1, space="PSUM") as ps_pq:
                for ci, (c0, c1) in enumerate(CH):
                    pq = ps_pq.tile([1, c1 - c0], F32, tag=f"pq{ci}",
                                    name=f"pq{ci}")
                    nc.tensor.matmul(pq[:], ones8[0:DS, :], sq8[0:DS, c0:c1],
                                     start=True, stop=True)
                    nc.vector.tensor_scalar(psq8[0:1, c0:c1], pq[:],
                                            -0.25, None, ALU.mult)

            nc.sync.dma_start(ag_in[0:DS, :], pr2[0:DS, :])
            nc.sync.dma_start(ag_in[DS : DS + 1, :], psq8[:])
            nc.gpsimd.collective_compute(
                "AllGather", ALU.bypass,
                replica_groups=[list(range(W))],
                ins=[ag_in.opt()], outs=[ag_out.opt()],
            )

        # ---- emb transposes (fp8, stride-2 psum) fill the collective gap ----
        with tc.tile_pool(name="ps_tr", bufs=6, space="PSUM") as ps_tr:
            for t in range(KT):
                if t == 44:
                    _psq_ag()
                trb = ps_tr.tile([P, 2 * D], FP8, tag="trb", name="trb")
                trv = trb.rearrange("p (c two) -> p c two", two=2)
                for dc in range(DC):
                    nc.tensor.matmul(
                        trv[:, dc * P : (dc + 1) * P, 0:1],
                        e8[:, t * D + dc * P : t * D + (dc + 1) * P],
                        ident_8[:],
                        is_transpose=True,
                        start=(dc == 0), stop=(dc == DC - 1),
                    )
                dst8 = embT8[:, t * D : (t + 1) * D]
                nc.scalar.copy(dst8, trv[:, 0 : D, 0])


        # protos2T blocks land pre-transposed: block k rows -> chunk layout
        for k in range(W):
            dcq, off = (k * DS) // P, (k * DS) % P
            nc.sync.dma_start(
                protosT8[off : off + DS, dcq * C : (dcq + 1) * C],
                ag_out[k * AGB : k * AGB + DS, :],
            )
        # psq partial rows -> [8, C] tile, summed via ones-matmul
        psqs = cpool.tile([8, C], FP8, name="psqs")
        nc.sync.dma_start(
            psqs[:],
            ag_out.rearrange("(k b) c -> k b c", b=AGB)[:, DS, :],
        )
        with tc.tile_pool(name="ps_pf", bufs=1, space="PSUM") as ps_pf:
            for ci, (c0, c1) in enumerate(CH):
                pf = ps_pf.tile([1, c1 - c0], F32, tag=f"pf{ci}",
                                name=f"pf{ci}")
                nc.tensor.matmul(pf[:], ones8[0:8, :], psqs[:, c0:c1],
                                 start=True, stop=True)
                nc.scalar.copy(fold_rhs[0:1, c0:c1], pf[:])
                lo = cpool.tile([1, c1 - c0], F32, tag=f"lo{ci}",
                                name=f"lo{ci}")
                nc.vector.tensor_tensor(out=lo[:], in0=pf[:],
                                        in1=fold_rhs[0:1, c0:c1],
                                        op=ALU.subtract)
                nc.vector.tensor_copy(
                    out=fold_rhs[0:1, C + c0 : C + c1], in_=lo[:])

        # ================= phase 2 =================
        fones_v = fold_ones.rearrange("p (pl m) -> p pl m", pl=2)
        frhs_v = fold_rhs.rearrange("p (pl c) -> p pl c", pl=2)
        with tc.tile_pool(name="ps_cr", bufs=3, space="PSUM") as ps_cr:
            for nt in range(KT):
                ot = out_pool.tile([P, C], F16, tag="ot", name="ot")
                for ci, (c0, c1) in enumerate(CH):
                    cr = ps_cr.tile([P, c1 - c0], F32, tag=f"cr{ci}",
                                    name=f"cr{ci}")
                    nc.tensor.matmul(
                        cr[:], fones_v[:, :, :], frhs_v[:, :, c0:c1],
                        start=True, stop=False, perf_mode=DR,
                    )
                    for pr in range(2):
                        lhs = embT8[
                            :, nt * D + pr * 2 * P : nt * D + (pr + 1) * 2 * P
                        ].rearrange("p (pl m) -> p pl m", pl=2)
                        rhs = protosT8[
                            :, 2 * pr * C : (2 * pr + 2) * C
                        ].rearrange("p (pl c) -> p pl c", pl=2)[:, :, c0:c1]
                        nc.tensor.matmul(
                            cr[:], lhs, rhs,
                            start=False, stop=(pr == 1),
                            perf_mode=DR,
                        )
                    if (2 * nt + ci) % 2 == 0:
                        nc.scalar.activation(
                            ot[:, c0:c1], cr[:], ACTF.Identity,
                            bias=esq_neg[:, nt : nt + 1], scale=1.0,
                        )
                    else:
                        nc.vector.tensor_scalar(
                            ot[:, c0:c1], cr[:], esq_neg[:, nt : nt + 1],
                            None, ALU.add,
                        )
                st_eng = nc.sync if nt % 3 < 2 else nc.gpsimd
                st_eng.dma_start(out_ext[nt * P : (nt + 1) * P, :], ot[:])

    _split_waits(nc)
    return nc


def kernel(embeddings, prototypes, counter, y_true):
    embeddings = np.ascontiguousarray(np.asarray(embeddings, dtype=np.float32))
    prototypes = np.ascontiguousarray(np.asarray(prototypes, dtype=np.float32))
    counter_f = np.asarray(counter, dtype=np.float64)
    y = np.asarray(y_true).astype(np.int64)

    # host-side: counts + running-mean coefficients (index math only)
    counts = np.bincount(y, minlength=C).astype(np.float64)
    rep = counts > 0
    rm = 1.0 / np.maximum(counts, 1.0)
    rt = 1.0 / (counter_f + 1.0)
    B2 = (2.0 * rep * rm * rt).astype(np.float32)
    A2 = (2.0 * (1.0 + rep * (counter_f * rt - 1.0))).astype(np.float32)
    p0T = prototypes.T  # [D, C]

    if _built[0] is None:
        _built[0] = _build()
    nc = _built[0]

    in_maps = []
    for i in range(W):
        sl = slice(i * NL, (i + 1) * NL)
        ds = slice(i * DS, (i + 1) * DS)
        y_loc = y[sl].astype(np.float32)
        yf = np.ascontiguousarray(y_loc.reshape(KT, P).T)
        in_maps.append(
            {
                "emb": embeddings[sl],
                "yf": yf,
                "ap0": np.ascontiguousarray(A2[None, :] * p0T[ds]),
                "bb": np.ascontiguousarray(
                    np.broadcast_to(B2[None, :], (DS, C))
                ),
            }
        )

    res = run_bass_kernel_spmd(
        nc, in_maps, list(range(W)), trace=PROFILE, **TRACE_KWARGS
    )
    LAST_RESULT[0] = res
    out = np.concatenate([res.results[i]["out"] for i in range(W)], axis=0)
    return out.astype(np.float32)


# revision 41
# speedup vs baseline: 2.1811x; 1.0479x over previous
"""DeepNCM Trainium2 kernel v3: fp8 DoubleRow one-hot segment sums +
fp8 DoubleRow distance GEMM, data-parallel over embedding rows across 8 cores.

Contract: kernel(**inputs) takes the FULL unsharded inputs
(embeddings [65536,512] f32, prototypes [1000,512] f32, counter [1000] f32,
y_true [65536] int64) and returns the FULL output [65536,1000] f32.

Per-core plan (NL = 8192 rows):
  Host precomputes counts = bincount(y) and folds the running-mean update
  into per-class coefficients: protos2 = A2*p0 + B2*sums (factor 2 folded).
  The host also ships A2*p0^T and broadcast B2 so the device applies them
  with two elementwise ops.
  Phase 1 (per pair of 128-row tiles): DMA f32 emb; quantize to fp8;
  ACT computes e_sq (Square+accum from f32); DVE/Pool build fp8 one-hot
  tiles; PE accumulates sumsT[d,c] += emb^T @ onehot with DoubleRow fp8
  matmuls (two row-tiles per instruction). PSUM sums -> bf16 -> DRAM.
  ReduceScatter gives each core a 64-row D-slice of the reduced sumsT;
  it computes its protos2T slice + a -p_sq/4 partial row, quantizes to
  fp8, and an AllGather replicates [8*(64+1), 1000] fp8 — already in the
  [D, C] layout phase 2 needs. p_sq partials are summed with a tiny
  ones-matmul into the K=1 fold rows.
  Phase 2: cross via fp8 DoubleRow matmuls + a K=1 DoubleRow instruction
  folding -p_sq into PSUM; ACT/DVE epilogue adds -e_sq (per-partition
  bias) and writes fp16; host upcasts. Emb transposes (PE, fp8, stride-2
  PSUM) and their SBUF copies run inside the collective window.
"""

import os
import sys
from contextlib import ExitStack

for _p in ("/opt/trn_rl_repo", "/root/.axon_site/_ro/trn_rl_repo"):
    if os.path.isdir(_p):
        if _p not in sys.path:
            sys.path.insert(0, _p)
        break

import numpy as np

import concourse.bass as bass
import concourse.mybir as mybir
import concourse.tile as tile
from concourse.masks import make_identity
from concourse.bass_utils import run_bass_kernel_spmd

N, D, C = 65536, 512, 1000
W = 8                      # cores
NL = N // W                # rows per core
P = 128
KT = NL // P               # 64 row tiles per core
DC = D // P                # 4 D chunks of 128
DS = D // W                # 64-row D-slice per core after ReduceScatter
AGB = DS + 1               # AllGather block: 64 protos2T rows + 1 psq row
CH = ((0, 512), (512, 1000))   # free-dim halves of the class axis
F32 = mybir.dt.float32
F16 = mybir.dt.float16
BF16 = mybir.dt.bfloat16
FP8 = mybir.dt.float8e4
ALU = mybir.AluOpType
ACTF = mybir.ActivationFunctionType
DR = mybir.MatmulPerfMode.DoubleRow

# Toggled by test.py for profiling runs.
PROFILE = False
TRACE_KWARGS = {}
LAST_RESULT = [None]

_built = [None]


def _split_waits(nc, cap=1):
    """Walrus in this container rejects >1 sync-wait per instruction.
    Move excess waits onto preceding same-engine NOPs (in-order engines,
    so semantics are preserved)."""
    n_new = 0
    for fn in nc.m.functions:
        for bb in fn.blocks:
            new_list = []
            for ins in bb.instructions:
                si = getattr(ins, "sync_info", None)
                if si is not None and si.on_wait and len(si.on_wait) > cap:
                    waits = list(si.on_wait)
                    keep, rest = waits[:cap], waits[cap:]
                    for i in range(0, len(rest), cap):
                        nop = mybir.InstNoOp(
                            name=f"I-waitsplit-{n_new}", ins=[], outs=[]
                        )
                        n_new += 1
                        nop.engine = ins.engine
                        nop.sync_info = mybir.SyncInfo(
                            on_wait=rest[i : i + cap], on_update=[]
                        )
                        new_list.append(nop)
                    si.on_wait = keep
                new_list.append(ins)
            bb.instructions = new_list
    return n_new


def _build():
    nc = bass.Bass()
    emb_ext = nc.declare_dram_parameter("emb", [NL, D], F32, isOutput=False)
    yf_ext = nc.declare_dram_parameter("yf", [P, KT], F32, isOutput=False)
    ap0_ext = nc.declare_dram_parameter("ap0", [DS, C], F32, isOutput=False)
    bb_ext = nc.declare_dram_parameter("bb", [DS, C], F32, isOutput=False)
    out_ext = nc.declare_dram_parameter("out", [NL, C], F16, isOutput=True)

    with tile.TileContext(nc) as tc, ExitStack() as es:
        cpool = es.enter_context(tc.tile_pool(name="const", bufs=1))
        bpool = es.enter_context(tc.tile_pool(name="bigs", bufs=1))
        in_pool = es.enter_context(tc.tile_pool(name="inp", bufs=16))
        oh_pool = es.enter_context(tc.tile_pool(name="oh", bufs=4))
        sq_pool = es.enter_context(tc.tile_pool(name="sq", bufs=2))
        out_pool = es.enter_context(tc.tile_pool(name="outp", bufs=6))
        dram = es.enter_context(tc.tile_pool(name="dram", bufs=1, space="DRAM"))

        # ---- constants ----
        ident_8 = cpool.tile([P, P], FP8, name="ident8")
        make_identity(nc, ident_8[:])
        fold_ones = cpool.tile([1, 2 * P], FP8, name="fones")
        nc.vector.memset(fold_ones[:], 1.0)
        ones8 = cpool.tile([P, 1], FP8, name="ones8")
        nc.vector.memset(ones8[:], 1.0)
        iota = cpool.tile([P, C], F32, name="iota")
        nc.gpsimd.iota(
            iota[:], pattern=[[1, C]], base=0, channel_multiplier=0,
            allow_small_or_imprecise_dtypes=True,
        )
        y_sb = cpool.tile([P, KT], F32, name="y")
        nc.sync.dma_start(y_sb[:], yf_ext[:])
        ap0_sb = cpool.tile([P, C], F32, name="ap0")
        nc.sync.dma_start(ap0_sb[0:DS, :], ap0_ext[:])
        bb_sb = cpool.tile([P, C], F32, name="bb")
        nc.sync.dma_start(bb_sb[0:DS, :], bb_ext[:])

        esq_neg = cpool.tile([P, KT], F32, name="esqn")
        e8 = bpool.tile([P, KT * D], FP8, name="e8")
        embT8 = bpool.tile([P, KT * D], FP8, name="embT8")
        protosT8 = bpool.tile([P, DC * C], FP8, name="protosT8")
        fold_rhs = bpool.tile([1, 2 * C], FP8, name="foldr")

        deferred_esq = []
        # ================= phase 1: one-hot segment sums =================
        with tc.tile_pool(name="ps_sums", bufs=1, space="PSUM") as ps_sums:
            s_ps = [
                [ps_sums.tile([P, c1 - c0], F32, tag=f"s{dc}_{ci}",
                              name=f"s{dc}_{ci}")
                 for ci, (c0, c1) in enumerate(CH)]
                for dc in range(DC)
            ]
            for kp in range(KT // 2):
                t0 = kp * 2
                et = in_pool.tile([P, 2 * D], F32, tag="et", name="et")
                src = emb_ext[t0 * P : (t0 + 2) * P, :].rearrange(
                    "(i p) d -> p i d", i=2
                )
                ld_eng = nc.sync if kp % 4 != 1 else nc.gpsimd
                ld_eng.dma_start(et.rearrange("p (i d) -> p i d", i=2), src)
                # fp8 quantization (feeds the sums matmuls AND phase 2)
                dst = e8[:, t0 * D : (t0 + 2) * D]
                if kp % 2 == 0:
                    nc.gpsimd.tensor_copy(out=dst, in_=et[:])
                else:
                    nc.vector.tensor_copy(out=dst, in_=et[:])
                # e_sq from f32 (exact); deferred for the last 16 pairs so
                # ACT can drain the transpose copies first
                if kp < KT // 4:
                    for i in range(2):
                        scr = sq_pool.tile([P, D], BF16, tag="scr", name="scr")
                        nc.scalar.activation(
                            scr[:], et[:, i * D : (i + 1) * D], ACTF.Square,
                            accum_out=esq_neg[:, t0 + i : t0 + i + 1],
                        )
                else:
                    deferred_esq.append((et, t0))
                # one-hot pair tile [128, 2, C] fp8
                oh = oh_pool.tile([P, 2 * C], FP8, tag="oh", name="oh")
                for i in range(2):
                    oh_eng = nc.vector if (kp + i) % 3 != 2 else nc.gpsimd
                    oh_eng.tensor_scalar(
                        oh[:, i * C : (i + 1) * C], iota[:],
                        y_sb[:, t0 + i : t0 + i + 1], None, ALU.is_equal,
                    )
                ohv = oh.rearrange("p (pl c) -> p pl c", pl=2)
                e8v = e8.rearrange("p (nt dc m) -> p nt dc m", nt=KT, dc=DC)
                for dc in range(DC):
                    lhs = e8v[:, t0 : t0 + 2, dc, :]  # [P, 2, 128]
                    for ci, (c0, c1) in enumerate(CH):
                        nc.tensor.matmul(
                            s_ps[dc][ci][:],
                            lhs,
                            ohv[:, :, c0:c1],
                            start=(kp == 0), stop=(kp == KT // 2 - 1),
                            perf_mode=DR,
                        )
            # sums psum -> sbuf bf16 (D-major [512, 1000])
            sums_sb = cpool.tile([P, DC * C], BF16, name="sumssb")
            for dc in range(DC):
                for ci, (c0, c1) in enumerate(CH):
                    dsts = sums_sb[:, dc * C + c0 : dc * C + c1]
                    nc.vector.tensor_copy(out=dsts, in_=s_ps[dc][ci][:])

        sums_d = dram.tile([D, C], BF16, name="sumsd")
        for dc in range(DC):
            (nc.sync if dc % 2 == 0 else nc.gpsimd).dma_start(
                sums_d[dc * P : (dc + 1) * P, :],
                sums_sb[:, dc * C : (dc + 1) * C],
            )

        # ---- ReduceScatter: core i owns D rows [64i, 64i+64) ----
        rs_out = dram.tile([DS, C], BF16, name="rsout")
        nc.gpsimd.collective_compute(
            "ReduceScatter", ALU.add,
            replica_groups=[list(range(W))],
            ins=[sums_d.opt()], outs=[rs_out.opt()],
        )
        sums_rs = cpool.tile([P, C], BF16, name="sumsrs")
        nc.sync.dma_start(sums_rs[0:DS, :], rs_out[:])

        # ---- protos2T slice + psq partial, quantize, AllGather ----
        pr2 = cpool.tile([P, C], FP8, name="pr2")
        t2 = cpool.tile([P, C], F32, name="t2")
        nc.vector.tensor_tensor(out=t2[0:DS, :], in0=sums_rs[0:DS, :],
                                in1=bb_sb[0:DS, :], op=ALU.mult)
        nc.vector.tensor_tensor(out=pr2[0:DS, :], in0=t2[0:DS, :],
                                in1=ap0_sb[0:DS, :], op=ALU.add)
        sq8 = cpool.tile([P, C], FP8, name="sq8")
        nc.vector.tensor_tensor(out=sq8[0:DS, :], in0=pr2[0:DS, :],
                                in1=pr2[0:DS, :], op=ALU.mult)

        ag_in = dram.tile([AGB, C], FP8, name="agin")
        ag_out = dram.tile([W * AGB, C], FP8, name="agout",
                           addr_space="Shared")

        def _psq_ag():
            psq8 = cpool.tile([1, C], FP8, name="psq8")
            with tc.tile_pool(name="ps_pq", bufs=1, space="PSUM") as ps_pq:
                for ci, (c0, c1) in enumerate(CH):
                    pq = ps_pq.tile([1, c1 - c0], F32, tag=f"pq{ci}",
                                    name=f"pq{ci}")
                    nc.tensor.matmul(pq[:], ones8[0:DS, :], sq8[0:DS, c0:c1],
                                     start=True, stop=True)
                    nc.vector.tensor_scalar(psq8[0:1, c0:c1], pq[:],
                                            -0.25, None, ALU.mult)

            nc.sync.dma_start(ag_in[0:DS, :], pr2[0:DS, :])
            nc.sync.dma_start(ag_in[DS : DS + 1, :], psq8[:])
            nc.gpsimd.collective_compute(
                "AllGather", ALU.bypass,
                replica_groups=[list(range(W))],
                ins=[ag_in.opt()], outs=[ag_out.opt()],
            )

        # ---- emb transposes (fp8, stride-2 psum) fill the collective gap ----
        with tc.tile_pool(name="ps_tr", bufs=6, space="PSUM") as ps_tr:
            for t in range(KT):
                if t == 44:
                    _psq_ag()
                trb = ps_tr.tile([P, 2 * D], FP8, tag="trb", name="trb")
                trv = trb.rearrange("p (c two) -> p c two", two=2)
                for dc in range(DC):
                    nc.tensor.matmul(
                        trv[:, dc * P : (dc + 1) * P, 0:1],
                        e8[:, t * D + dc * P : t * D + (dc + 1) * P],
                        ident_8[:],
                        is_transpose=True,
                        start=(dc == 0), stop=(dc == DC - 1),
                    )
                dst8 = embT8[:, t * D : (t + 1) * D]
                nc.scalar.copy(dst8, trv[:, 0 : D, 0])


        # protos2T blocks land pre-transposed: block k rows -> chunk layout
        for k in range(W):
            dcq, off = (k * DS) // P, (k * DS) % P
            nc.sync.dma_start(
                protosT8[off : off + DS, dcq * C : (dcq + 1) * C],
                ag_out[k * AGB : k * AGB + DS, :],
            )
        # psq partial rows -> [8, C] tile, summed via ones-matmul
        psqs = cpool.tile([8, C], FP8, name="psqs")
        nc.sync.dma_start(
            psqs[:],
            ag_out.rearrange("(k b) c -> k b c", b=AGB)[:, DS, :],
        )
        with tc.tile_pool(name="ps_pf", bufs=1, space="PSUM") as ps_pf:
            for ci, (c0, c1) in enumerate(CH):
                pf = ps_pf.tile([1, c1 - c0], F32, tag=f"pf{ci}",
                                name=f"pf{ci}")
                nc.tensor.matmul(pf[:], ones8[0:8, :], psqs[:, c0:c1],
                                 start=True, stop=True)
                nc.vector.tensor_copy(out=fold_rhs[0:1, c0:c1],
                                      in_=pf[:])
                lo = cpool.tile([1, c1 - c0], F32, tag=f"lo{ci}",
                                name=f"lo{ci}")
                nc.vector.tensor_tensor(out=lo[:], in0=pf[:],
                                        in1=fold_rhs[0:1, c0:c1],
                                        op=ALU.subtract)
                nc.vector.tensor_copy(
                    out=fold_rhs[0:1, C + c0 : C + c1], in_=lo[:])

        for et_d, t0 in deferred_esq:
            for i in range(2):
                scr = sq_pool.tile([P, D], BF16, tag="scr", name="scr")
                nc.scalar.activation(
                    scr[:], et_d[:, i * D : (i + 1) * D], ACTF.Square,
                    accum_out=esq_neg[:, t0 + i : t0 + i + 1],
                )

        # negate e_sq once (used as ScalarE bias in phase 2)
        nc.vector.tensor_scalar(esq_neg[:], esq_neg[:], -1.0, None, ALU.mult)

        # ================= phase 2 =================
        fones_v = fold_ones.rearrange("p (pl m) -> p pl m", pl=2)
        frhs_v = fold_rhs.rearrange("p (pl c) -> p pl c", pl=2)
        with tc.tile_pool(name="ps_cr", bufs=4, space="PSUM") as ps_cr:
            for nt in range(KT):
                ot = out_pool.tile([P, C], F16, tag="ot", name="ot")
                for ci, (c0, c1) in enumerate(CH):
                    cr = ps_cr.tile([P, c1 - c0], F32, tag=f"cr{ci}",
                                    name=f"cr{ci}")
                    nc.tensor.matmul(
                        cr[:], fones_v[:, :, :], frhs_v[:, :, c0:c1],
                        start=True, stop=False, perf_mode=DR,
                    )
                    for pr in range(2):
                        lhs = embT8[
                            :, nt * D + pr * 2 * P : nt * D + (pr + 1) * 2 * P
                        ].rearrange("p (pl m) -> p pl m", pl=2)
                        rhs = protosT8[
                            :, 2 * pr * C : (2 * pr + 2) * C
                        ].rearrange("p (pl c) -> p pl c", pl=2)[:, :, c0:c1]
                        nc.tensor.matmul(
                            cr[:], lhs, rhs,
                            start=False, stop=(pr == 1),
                            perf_mode=DR,
                        )
                    if (2 * nt + ci) % 2 == 0:
                        nc.scalar.activation(
                            ot[:, c0:c1], cr[:], ACTF.Identity,
                            bias=esq_neg[:, nt : nt + 1], scale=1.0,
                        )
                    else:
                        nc.vector.tensor_scalar(
                            ot[:, c0:c1], cr[:], esq_neg[:, nt : nt + 1],
                            None, ALU.add,
                        )
                st_eng = nc.sync if nt % 3 < 2 else nc.gpsimd
                st_eng.dma_start(out_ext[nt * P : (nt + 1) * P, :], ot[:])

    _split_waits(nc)
    return nc


def kernel(embeddings, prototypes, counter, y_true):
    embeddings = np.ascontiguousarray(np.asarray(embeddings, dtype=np.float32))
    prototypes = np.ascontiguousarray(np.asarray(prototypes, dtype=np.float32))
    counter_f = np.asarray(counter, dtype=np.float64)
    y = np.asarray(y_true).astype(np.int64)

    # host-side: counts + running-mean coefficients (index math only)
    counts = np.bincount(y, minlength=C).astype(np.float64)
    rep = counts > 0
    rm = 1.0 / np.maximum(counts, 1.0)
    rt = 1.0 / (counter_f + 1.0)
    B2 = (2.0 * rep * rm * rt).astype(np.float32)
    A2 = (2.0 * (1.0 + rep * (counter_f * rt - 1.0))).astype(np.float32)
    p0T = prototypes.T  # [D, C]

    if _built[0] is None:
        _built[0] = _build()
    nc = _built[0]

    in_maps = []
    for i in range(W):
        sl = slice(i * NL, (i + 1) * NL)
        ds = slice(i * DS, (i + 1) * DS)
        y_loc = y[sl].astype(np.float32)
        yf = np.ascontiguousarray(y_loc.reshape(KT, P).T)
        in_maps.append(
            {
                "emb": embeddings[sl],
                "yf": yf,
                "ap0": np.ascontiguousarray(A2[None, :] * p0T[ds]),
                "bb": np.ascontiguousarray(
                    np.broadcast_to(B2[None, :], (DS, C))
                ),
            }
        )

    res = run_bass_kernel_spmd(
        nc, in_maps, list(range(W)), trace=PROFILE, **TRACE_KWARGS
    )
    LAST_RESULT[0] = res
    out = np.concatenate([res.results[i]["out"] for i in range(W)], axis=0)
    return out.astype(np.float32)


# revision 46
# speedup vs baseline: 2.2153x; 1.0157x over previous
"""DeepNCM Trainium2 kernel v3: fp8 DoubleRow one-hot segment sums +
fp8 DoubleRow distance GEMM, data-parallel over embedding rows across 8 cores.

Contract: kernel(**inputs) takes the FULL unsharded inputs
(embeddings [65536,512] f32, prototypes [1000,512] f32, counter [1000] f32,
y_true [65536] int64) and returns the FULL output [65536,1000] f32.

Per-core plan (NL = 8192 rows):
  Host precomputes counts = bincount(y) and folds the running-mean update
  into per-class coefficients: protos2 = A2*p0 + B2*sums (factor 2 folded).
  The host also ships A2*p0^T and broadcast B2 so the device applies them
  with two elementwise ops.
  Phase 1 (per pair of 128-row tiles): DMA f32 emb; quantize to fp8;
  ACT computes e_sq (Square+accum from f32); DVE/Pool build fp8 one-hot
  tiles; PE accumulates sumsT[d,c] += emb^T @ onehot with DoubleRow fp8
  matmuls (two row-tiles per instruction). PSUM sums -> bf16 -> DRAM.
  ReduceScatter gives each core a 64-row D-slice of the reduced sumsT;
  it computes its protos2T slice + a -p_sq/4 partial row, quantizes to
  fp8, and an AllGather replicates [8*(64+1), 1000] fp8 — already in the
  [D, C] layout phase 2 needs. p_sq partials are summed with a tiny
  ones-matmul into the K=1 fold rows.
  Phase 2: cross via fp8 DoubleRow matmuls + a K=1 DoubleRow instruction
  folding -p_sq into PSUM; ACT/DVE epilogue adds -e_sq (per-partition
  bias) and writes fp16; host upcasts. Emb transposes (PE, fp8, stride-2
  PSUM) and their SBUF copies run inside the collective window.
"""

import os
import sys
from contextlib import ExitStack

for _p in ("/opt/trn_rl_repo", "/root/.axon_site/_ro/trn_rl_repo"):
    if os.path.isdir(_p):
        if _p not in sys.path:
            sys.path.insert(0, _p)
        break

import numpy as np

import concourse.bass as bass
import concourse.mybir as mybir
import concourse.tile as tile
from concourse.masks import make_identity
from concourse.bass_utils import run_bass_kernel_spmd

N, D, C = 65536, 512, 1000
W = 8                      # cores
NL = N // W                # rows per core
P = 128
KT = NL // P               # 64 row tiles per core
DC = D // P                # 4 D chunks of 128
DS = D // W                # 64-row D-slice per core after ReduceScatter
AGB = DS + 1               # AllGather block: 64 protos2T rows + 1 psq row
CH = ((0, 512), (512, 1000))   # free-dim halves of the class axis
F32 = mybir.dt.float32
F16 = mybir.dt.float16
BF16 = mybir.dt.bfloat16
FP8 = mybir.dt.float8e4
ALU = mybir.AluOpType
ACTF = mybir.ActivationFunctionType
DR = mybir.MatmulPerfMode.DoubleRow

# Toggled by test.py for profiling runs.
PROFILE = False
TRACE_KWARGS = {}
LAST_RESULT = [None]

_built = [None]


def _split_waits(nc, cap=1):
    """Walrus in this container rejects >1 sync-wait per instruction.
    Move excess waits onto preceding same-engine NOPs (in-order engines,
    so semantics are preserved)."""
    n_new = 0
    for fn in nc.m.functions:
        for bb in fn.blocks:
            new_list = []
            for ins in bb.instructions:
                si = getattr(ins, "sync_info", None)
                if si is not None and si.on_wait and len(si.on_wait) > cap:
                    waits = list(si.on_wait)
                    keep, rest = waits[:cap], waits[cap:]
                    for i in range(0, len(rest), cap):
                        nop = mybir.InstNoOp(
                            name=f"I-waitsplit-{n_new}", ins=[], outs=[]
                        )
                        n_new += 1
                        nop.engine = ins.engine
                        nop.sync_info = mybir.SyncInfo(
                            on_wait=rest[i : i + cap], on_update=[]
                        )
                        new_list.append(nop)
                    si.on_wait = keep
                new_list.append(ins)
            bb.instructions = new_list
    return n_new


def _build():
    nc = bass.Bass()
    emb_ext = nc.declare_dram_parameter("emb", [NL, D], F32, isOutput=False)
    yf_ext = nc.declare_dram_parameter("yf", [P, KT], F32, isOutput=False)
    ap0_ext = nc.declare_dram_parameter("ap0", [DS, C], F32, isOutput=False)
    bb_ext = nc.declare_dram_parameter("bb", [DS, C], F32, isOutput=False)
    out_ext = nc.declare_dram_parameter("out", [NL, C], F16, isOutput=True)

    with tile.TileContext(nc) as tc, ExitStack() as es:
        cpool = es.enter_context(tc.tile_pool(name="const", bufs=1))
        bpool = es.enter_context(tc.tile_pool(name="bigs", bufs=1))
        in_pool = es.enter_context(tc.tile_pool(name="inp", bufs=16))
        oh_pool = es.enter_context(tc.tile_pool(name="oh", bufs=4))
        sq_pool = es.enter_context(tc.tile_pool(name="sq", bufs=2))
        out_pool = es.enter_context(tc.tile_pool(name="outp", bufs=6))
        dram = es.enter_context(tc.tile_pool(name="dram", bufs=1, space="DRAM"))

        # ---- constants ----
        ident_8 = cpool.tile([P, P], FP8, name="ident8")
        make_identity(nc, ident_8[:])
        fold_ones = cpool.tile([1, 2 * P], FP8, name="fones")
        nc.vector.memset(fold_ones[:], 1.0)
        ones8 = cpool.tile([P, 1], FP8, name="ones8")
        nc.vector.memset(ones8[:], 1.0)
        iota = cpool.tile([P, C], F32, name="iota")
        nc.gpsimd.iota(
            iota[:], pattern=[[1, C]], base=0, channel_multiplier=0,
            allow_small_or_imprecise_dtypes=True,
        )
        y_sb = cpool.tile([P, KT], F32, name="y")
        nc.sync.dma_start(y_sb[:], yf_ext[:])
        ap0_sb = cpool.tile([P, C], F32, name="ap0")
        nc.sync.dma_start(ap0_sb[0:DS, :], ap0_ext[:])
        bb_sb = cpool.tile([P, C], F32, name="bb")
        nc.sync.dma_start(bb_sb[0:DS, :], bb_ext[:])

        esq_neg = cpool.tile([P, KT], F32, name="esqn")
        e8 = bpool.tile([P, KT * D], FP8, name="e8")
        embT8 = bpool.tile([P, KT * D], FP8, name="embT8")
        protosT8 = bpool.tile([P, DC * C], FP8, name="protosT8")
        fold_rhs = bpool.tile([1, 2 * C], FP8, name="foldr")

        deferred_esq = []
        # ================= phase 1: one-hot segment sums =================
        with tc.tile_pool(name="ps_sums", bufs=1, space="PSUM") as ps_sums:
            s_ps = [
                [ps_sums.tile([P, c1 - c0], F32, tag=f"s{dc}_{ci}",
                              name=f"s{dc}_{ci}")
                 for ci, (c0, c1) in enumerate(CH)]
                for dc in range(DC)
            ]
            for kp in range(KT // 2):
                t0 = kp * 2
                et = in_pool.tile([P, 2 * D], F32, tag="et", name="et")
                src = emb_ext[t0 * P : (t0 + 2) * P, :].rearrange(
                    "(i p) d -> p i d", i=2
                )
                ld_eng = nc.sync if kp % 4 != 1 else nc.gpsimd
                ld_eng.dma_start(et.rearrange("p (i d) -> p i d", i=2), src)
                # fp8 quantization (feeds the sums matmuls AND phase 2)
                dst = e8[:, t0 * D : (t0 + 2) * D]
                if kp % 2 == 0:
                    nc.gpsimd.tensor_copy(out=dst, in_=et[:])
                else:
                    nc.vector.tensor_copy(out=dst, in_=et[:])
                # e_sq from f32 (exact); deferred for the last 16 pairs so
                # ACT can drain the transpose copies first
                if kp < KT // 4:
                    for i in range(2):
                        scr = sq_pool.tile([P, D], BF16, tag="scr", name="scr")
                        nc.scalar.activation(
                            scr[:], et[:, i * D : (i + 1) * D], ACTF.Square,
                            accum_out=esq_neg[:, t0 + i : t0 + i + 1],
                        )
                else:
                    deferred_esq.append((et, t0))
                # one-hot pair tile [128, 2, C] fp8
                oh = oh_pool.tile([P, 2 * C], FP8, tag="oh", name="oh")
                for i in range(2):
                    oh_eng = nc.vector if (kp + i) % 3 != 2 else nc.gpsimd
                    oh_eng.tensor_scalar(
                        oh[:, i * C : (i + 1) * C], iota[:],
                        y_sb[:, t0 + i : t0 + i + 1], None, ALU.is_equal,
                    )
                ohv = oh.rearrange("p (pl c) -> p pl c", pl=2)
                e8v = e8.rearrange("p (nt dc m) -> p nt dc m", nt=KT, dc=DC)
                for dc in range(DC):
                    lhs = e8v[:, t0 : t0 + 2, dc, :]  # [P, 2, 128]
                    for ci, (c0, c1) in enumerate(CH):
                        nc.tensor.matmul(
                            s_ps[dc][ci][:],
                            lhs,
                            ohv[:, :, c0:c1],
                            start=(kp == 0), stop=(kp == KT // 2 - 1),
                            perf_mode=DR,
                        )
            # sums psum -> sbuf bf16 (D-major [512, 1000])
            sums_sb = cpool.tile([P, DC * C], BF16, name="sumssb")
            for dc in range(DC):
                for ci, (c0, c1) in enumerate(CH):
                    dsts = sums_sb[:, dc * C + c0 : dc * C + c1]
                    if (dc + ci) % 2 == 0:
                        nc.scalar.copy(dsts, s_ps[dc][ci][:])
                    else:
                        nc.vector.tensor_copy(out=dsts, in_=s_ps[dc][ci][:])

        sums_d = dram.tile([D, C], BF16, name="sumsd")
        for dc in range(DC):
            (nc.sync if dc % 2 == 0 else nc.gpsimd).dma_start(
                sums_d[dc * P : (dc + 1) * P, :],
                sums_sb[:, dc * C : (dc + 1) * C],
            )

        # ---- ReduceScatter: core i owns D rows [64i, 64i+64) ----
        rs_out = dram.tile([DS, C], BF16, name="rsout")
        nc.gpsimd.collective_compute(
            "ReduceScatter", ALU.add,
            replica_groups=[list(range(W))],
            ins=[sums_d.opt()], outs=[rs_out.opt()],
        )
        sums_rs = cpool.tile([P, C], BF16, name="sumsrs")
        nc.sync.dma_start(sums_rs[0:DS, :], rs_out[:])

        # ---- protos2T slice + psq partial, quantize, AllGather ----
        pr2 = cpool.tile([P, C], FP8, name="pr2")
        t2 = cpool.tile([P, C], F32, name="t2")
        nc.vector.tensor_tensor(out=t2[0:DS, :], in0=sums_rs[0:DS, :],
                                in1=bb_sb[0:DS, :], op=ALU.mult)
        nc.vector.tensor_tensor(out=pr2[0:DS, :], in0=t2[0:DS, :],
                                in1=ap0_sb[0:DS, :], op=ALU.add)
        sq8 = cpool.tile([P, C], FP8, name="sq8")
        nc.vector.tensor_tensor(out=sq8[0:DS, :], in0=pr2[0:DS, :],
                                in1=pr2[0:DS, :], op=ALU.mult)

        ag_in = dram.tile([AGB, C], FP8, name="agin")
        ag_out = dram.tile([W * AGB, C], FP8, name="agout",
                           addr_space="Shared")

        def _psq_ag():
            psq8 = cpool.tile([1, C], FP8, name="psq8")
            with tc.tile_pool(name="ps_pq", bufs=1, space="PSUM") as ps_pq:
                for ci, (c0, c1) in enumerate(CH):
                    pq = ps_pq.tile([1, c1 - c0], F32, tag=f"pq{ci}",
                                    name=f"pq{ci}")
                    nc.tensor.matmul(pq[:], ones8[0:DS, :], sq8[0:DS, c0:c1],
                                     start=True, stop=True)
                    nc.vector.tensor_scalar(psq8[0:1, c0:c1], pq[:],
                                            -0.25, None, ALU.mult)

            nc.sync.dma_start(ag_in[0:DS, :], pr2[0:DS, :])
            nc.sync.dma_start(ag_in[DS : DS + 1, :], psq8[:])
            nc.gpsimd.collective_compute(
                "AllGather", ALU.bypass,
                replica_groups=[list(range(W))],
                ins=[ag_in.opt()], outs=[ag_out.opt()],
            )

        # ---- emb transposes (fp8, stride-2 psum) fill the collective gap ----
        with tc.tile_pool(name="ps_tr", bufs=6, space="PSUM") as ps_tr:
            for t in range(KT):
                if t == 44:
                    _psq_ag()
                trb = ps_tr.tile([P, 2 * D], FP8, tag="trb", name="trb")
                trv = trb.rearrange("p (c two) -> p c two", two=2)
                for dc in range(DC):
                    nc.tensor.matmul(
                        trv[:, dc * P : (dc + 1) * P, 0:1],
                        e8[:, t * D + dc * P : t * D + (dc + 1) * P],
                        ident_8[:],
                        is_transpose=True,
                        start=(dc == 0), stop=(dc == DC - 1),
                    )
                dst8 = embT8[:, t * D : (t + 1) * D]
                nc.scalar.copy(dst8, trv[:, 0 : D, 0])


        # protos2T blocks land pre-transposed: block k rows -> chunk layout
        agov = ag_out.rearrange("(dcq h b) c -> dcq h b c", dcq=DC, h=2)
        ptv = protosT8.rearrange("p (dcq c) -> p dcq c", dcq=DC)
        for h in range(2):
            nc.sync.dma_start(
                ptv[h * DS : (h + 1) * DS, :, :],
                agov[:, h, 0:DS, :].rearrange("dcq b c -> b dcq c"),
            )
        # psq partial rows -> [8, C] tile, summed via ones-matmul
        psqs = cpool.tile([8, C], FP8, name="psqs")
        nc.sync.dma_start(
            psqs[:],
            ag_out.rearrange("(k b) c -> k b c", b=AGB)[:, DS, :],
        )
        with tc.tile_pool(name="ps_pf", bufs=1, space="PSUM") as ps_pf:
            for ci, (c0, c1) in enumerate(CH):
                pf = ps_pf.tile([1, c1 - c0], F32, tag=f"pf{ci}",
                                name=f"pf{ci}")
                nc.tensor.matmul(pf[:], ones8[0:8, :], psqs[:, c0:c1],
                                 start=True, stop=True)
                nc.vector.tensor_copy(out=fold_rhs[0:1, c0:c1],
                                      in_=pf[:])
                lo = cpool.tile([1, c1 - c0], F32, tag=f"lo{ci}",
                                name=f"lo{ci}")
                nc.vector.tensor_tensor(out=lo[:], in0=pf[:],
                                        in1=fold_rhs[0:1, c0:c1],
                                        op=ALU.subtract)
                nc.vector.tensor_copy(
                    out=fold_rhs[0:1, C + c0 : C + c1], in_=lo[:])

        for et_d, t0 in deferred_esq:
            for i in range(2):
                scr = sq_pool.tile([P, D], BF16, tag="scr", name="scr")
                nc.scalar.activation(
                    scr[:], et_d[:, i * D : (i + 1) * D], ACTF.Square,
                    accum_out=esq_neg[:, t0 + i : t0 + i + 1],
                )

        # negate e_sq once (used as ScalarE bias in phase 2)
        nc.vector.tensor_scalar(esq_neg[:], esq_neg[:], -1.0, None, ALU.mult)

        # ================= phase 2 =================
        fones_v = fold_ones.rearrange("p (pl m) -> p pl m", pl=2)
        frhs_v = fold_rhs.rearrange("p (pl c) -> p pl c", pl=2)
        with tc.tile_pool(name="ps_cr", bufs=4, space="PSUM") as ps_cr:
            for nt in range(KT):
                ot = out_pool.tile([P, C], F16, tag="ot", name="ot")
                for ci, (c0, c1) in enumerate(CH):
                    cr = ps_cr.tile([P, c1 - c0], F32, tag=f"cr{ci}",
                                    name=f"cr{ci}")
                    nc.tensor.matmul(
                        cr[:], fones_v[:, :, :], frhs_v[:, :, c0:c1],
                        start=True, stop=False, perf_mode=DR,
                    )
                    for pr in range(2):
                        lhs = embT8[
                            :, nt * D + pr * 2 * P : nt * D + (pr + 1) * 2 * P
                        ].rearrange("p (pl m) -> p pl m", pl=2)
                        rhs = protosT8[
                            :, 2 * pr * C : (2 * pr + 2) * C
                        ].rearrange("p (pl c) -> p pl c", pl=2)[:, :, c0:c1]
                        nc.tensor.matmul(
                            cr[:], lhs, rhs,
                            start=False, stop=(pr == 1),
                            perf_mode=DR,
                        )
                    if (2 * nt + ci) % 2 == 0:
                        nc.scalar.activation(
                            ot[:, c0:c1], cr[:], ACTF.Identity,
                            bias=esq_neg[:, nt : nt + 1], scale=1.0,
                        )
                    else:
                        nc.vector.tensor_scalar(
                            ot[:, c0:c1], cr[:], esq_neg[:, nt : nt + 1],
                            None, ALU.add,
                        )
                st_eng = nc.sync if nt % 3 < 2 else nc.gpsimd
                st_eng.dma_start(out_ext[nt * P : (nt + 1) * P, :], ot[:])

    _split_waits(nc)
    return nc


def kernel(embeddings, prototypes, counter, y_true):
    embeddings = np.ascontiguousarray(np.asarray(embeddings, dtype=np.float32))
    prototypes = np.ascontiguousarray(np.asarray(prototypes, dtype=np.float32))
    counter_f = np.asarray(counter, dtype=np.float64)
    y = np.asarray(y_true).astype(np.int64)

    # host-side: counts + running-mean coefficients (index math only)
    counts = np.bincount(y, minlength=C).astype(np.float64)
    rep = counts > 0
    rm = 1.0 / np.maximum(counts, 1.0)
    rt = 1.0 / (counter_f + 1.0)
    B2 = (2.0 * rep * rm * rt).astype(np.float32)
    A2 = (2.0 * (1.0 + rep * (counter_f * rt - 1.0))).astype(np.float32)
    p0T = prototypes.T  # [D, C]

    if _built[0] is None:
        _built[0] = _build()
    nc = _built[0]

    in_maps = []
    for i in range(W):
        sl = slice(i * NL, (i + 1) * NL)
        ds = slice(i * DS, (i + 1) * DS)
        y_loc = y[sl].astype(np.float32)
        yf = np.ascontiguousarray(y_loc.reshape(KT, P).T)
        in_maps.append(
            {
                "emb": embeddings[sl],
                "yf": yf,
                "ap0": np.ascontiguousarray(A2[None, :] * p0T[ds]),
                "bb": np.ascontiguousarray(
                    np.broadcast_to(B2[None, :], (DS, C))
                ),
            }
        )

    res = run_bass_kernel_spmd(
        nc, in_maps, list(range(W)), trace=PROFILE, **TRACE_KWARGS
    )
    LAST_RESULT[0] = res
    out = np.concatenate([res.results[i]["out"] for i in range(W)], axis=0)
    return out.astype(np.float32)


# revision 50
# speedup vs baseline: 2.2764x; 1.0276x over previous
"""DeepNCM Trainium2 kernel v3: fp8 DoubleRow one-hot segment sums +
fp8 DoubleRow distance GEMM, data-parallel over embedding rows across 8 cores.

Contract: kernel(**inputs) takes the FULL unsharded inputs
(embeddings [65536,512] f32, prototypes [1000,512] f32, counter [1000] f32,
y_true [65536] int64) and returns the FULL output [65536,1000] f32.

Per-core plan (NL = 8192 rows):
  Host precomputes counts = bincount(y) and folds the running-mean update
  into per-class coefficients: protos2 = A2*p0 + B2*sums (factor 2 folded).
  The host also ships A2*p0^T and broadcast B2 so the device applies them
  with two elementwise ops.
  Phase 1 (per pair of 128-row tiles): DMA f32 emb; quantize to fp8;
  ACT computes e_sq (Square+accum from f32); DVE/Pool build fp8 one-hot
  tiles; PE accumulates sumsT[d,c] += emb^T @ onehot with DoubleRow fp8
  matmuls (two row-tiles per instruction). PSUM sums -> bf16 -> DRAM.
  ReduceScatter gives each core a 64-row D-slice of the reduced sumsT;
  it computes its protos2T slice + a -p_sq/4 partial row, quantizes to
  fp8, and an AllGather replicates [8*(64+1), 1000] fp8 — already in the
  [D, C] layout phase 2 needs. p_sq partials are summed with a tiny
  ones-matmul into the K=1 fold rows.
  Phase 2: cross via fp8 DoubleRow matmuls + a K=1 DoubleRow instruction
  folding -p_sq into PSUM; ACT/DVE epilogue adds -e_sq (per-partition
  bias) and writes fp16; host upcasts. Emb transposes (PE, fp8, stride-2
  PSUM) and their SBUF copies run inside the collective window.
"""

import os
import sys
from contextlib import ExitStack

for _p in ("/opt/trn_rl_repo", "/root/.axon_site/_ro/trn_rl_repo"):
    if os.path.isdir(_p):
        if _p not in sys.path:
            sys.path.insert(0, _p)
        break

import numpy as np

import concourse.bass as bass
import concourse.mybir as mybir
import concourse.tile as tile
from concourse.masks import make_identity
from concourse.bass_utils import run_bass_kernel_spmd

N, D, C = 65536, 512, 1000
W = 8                      # cores
NL = N // W                # rows per core
P = 128
KT = NL // P               # 64 row tiles per core
DC = D // P                # 4 D chunks of 128
DS = D // W                # 64-row D-slice per core after ReduceScatter
AGB = DS + 1               # AllGather block: 64 protos2T rows + 1 psq row
CH = ((0, 512), (512, 1000))   # free-dim halves of the class axis
F32 = mybir.dt.float32
F16 = mybir.dt.float16
BF16 = mybir.dt.bfloat16
FP8 = mybir.dt.float8e4
ALU = mybir.AluOpType
ACTF = mybir.ActivationFunctionType
DR = mybir.MatmulPerfMode.DoubleRow

# Toggled by test.py for profiling runs.
PROFILE = False
TRACE_KWARGS = {}
LAST_RESULT = [None]

_built = [None]


def _split_waits(nc, cap=1):
    """Walrus in this container rejects >1 sync-wait per instruction.
    Move excess waits onto preceding same-engine NOPs (in-order engines,
    so semantics are preserved)."""
    n_new = 0
    for fn in nc.m.functions:
        for bb in fn.blocks:
            new_list = []
            for ins in bb.instructions:
                si = getattr(ins, "sync_info", None)
                if si is not None and si.on_wait and len(si.on_wait) > cap:
                    waits = list(si.on_wait)
                    keep, rest = waits[:cap], waits[cap:]
                    for i in range(0, len(rest), cap):
                        nop = mybir.InstNoOp(
                            name=f"I-waitsplit-{n_new}", ins=[], outs=[]
                        )
                        n_new += 1
                        nop.engine = ins.engine
                        nop.sync_info = mybir.SyncInfo(
                            on_wait=rest[i : i + cap], on_update=[]
                        )
                        new_list.append(nop)
                    si.on_wait = keep
                new_list.append(ins)
            bb.instructions = new_list
    return n_new


def _build():
    nc = bass.Bass()
    emb_ext = nc.declare_dram_parameter("emb", [NL, D], F32, isOutput=False)
    yf_ext = nc.declare_dram_parameter("yf", [P, KT], F32, isOutput=False)
    ap0_ext = nc.declare_dram_parameter("ap0", [DS, C], F32, isOutput=False)
    bb_ext = nc.declare_dram_parameter("bb", [DS, C], F32, isOutput=False)
    out_ext = nc.declare_dram_parameter("out", [NL, C], F16, isOutput=True)

    with tile.TileContext(nc) as tc, ExitStack() as es:
        cpool = es.enter_context(tc.tile_pool(name="const", bufs=1))
        bpool = es.enter_context(tc.tile_pool(name="bigs", bufs=1))
        in_pool = es.enter_context(tc.tile_pool(name="inp", bufs=20))
        oh_pool = es.enter_context(tc.tile_pool(name="oh", bufs=4))
        sq_pool = es.enter_context(tc.tile_pool(name="sq", bufs=2))
        out_pool = es.enter_context(tc.tile_pool(name="outp", bufs=6))
        dram = es.enter_context(tc.tile_pool(name="dram", bufs=1, space="DRAM"))

        # ---- constants ----
        ident_8 = cpool.tile([P, P], FP8, name="ident8")
        make_identity(nc, ident_8[:])
        fold_ones = cpool.tile([1, 2 * P], FP8, name="fones")
        nc.vector.memset(fold_ones[:], 1.0)
        ones8 = cpool.tile([P, 1], FP8, name="ones8")
        nc.vector.memset(ones8[:], 1.0)
        iota = cpool.tile([P, C], F32, name="iota")
        nc.gpsimd.iota(
            iota[:], pattern=[[1, C]], base=0, channel_multiplier=0,
            allow_small_or_imprecise_dtypes=True,
        )
        y_sb = cpool.tile([P, KT], F32, name="y")
        nc.sync.dma_start(y_sb[:], yf_ext[:])
        ap0_sb = cpool.tile([P, C], F32, name="ap0")
        nc.sync.dma_start(ap0_sb[0:DS, :], ap0_ext[:])
        bb_sb = cpool.tile([P, C], F32, name="bb")
        nc.sync.dma_start(bb_sb[0:DS, :], bb_ext[:])

        esq_neg = cpool.tile([P, KT], F32, name="esqn")
        e8 = bpool.tile([P, KT * D], FP8, name="e8")
        embT8 = bpool.tile([P, KT * D], FP8, name="embT8")
        protosT8 = bpool.tile([P, DC * C], FP8, name="protosT8")
        fold_rhs = bpool.tile([1, 2 * C], FP8, name="foldr")
        nc.vector.memset(fold_rhs[0:1, C : 2 * C], 0.0)

        deferred_esq = []
        # ================= phase 1: one-hot segment sums =================
        with tc.tile_pool(name="ps_sums", bufs=1, space="PSUM") as ps_sums:
            s_ps = [
                [ps_sums.tile([P, c1 - c0], F32, tag=f"s{dc}_{ci}",
                              name=f"s{dc}_{ci}")
                 for ci, (c0, c1) in enumerate(CH)]
                for dc in range(DC)
            ]
            for kp in range(KT // 2):
                t0 = kp * 2
                et = in_pool.tile([P, 2 * D], F32, tag="et", name="et")
                src = emb_ext[t0 * P : (t0 + 2) * P, :].rearrange(
                    "(i p) d -> p i d", i=2
                )
                ld_eng = nc.sync if kp % 4 != 1 else nc.gpsimd
                ld_eng.dma_start(et.rearrange("p (i d) -> p i d", i=2), src)
                # fp8 quantization (feeds the sums matmuls AND phase 2)
                dst = e8[:, t0 * D : (t0 + 2) * D]
                if kp % 2 == 0:
                    nc.gpsimd.tensor_copy(out=dst, in_=et[:])
                else:
                    nc.vector.tensor_copy(out=dst, in_=et[:])
                # e_sq from f32 (exact); deferred for the last 16 pairs so
                # ACT can drain the transpose copies first
                if kp < 12:
                    for i in range(2):
                        scr = sq_pool.tile([P, D], BF16, tag="scr", name="scr")
                        nc.scalar.activation(
                            scr[:], et[:, i * D : (i + 1) * D], ACTF.Square,
                            accum_out=esq_neg[:, t0 + i : t0 + i + 1],
                        )
                else:
                    deferred_esq.append((et, t0))
                # one-hot pair tile [128, 2, C] fp8
                oh = oh_pool.tile([P, 2 * C], FP8, tag="oh", name="oh")
                for i in range(2):
                    oh_eng = nc.vector if (kp + i) % 3 != 2 else nc.gpsimd
                    oh_eng.tensor_scalar(
                        oh[:, i * C : (i + 1) * C], iota[:],
                        y_sb[:, t0 + i : t0 + i + 1], None, ALU.is_equal,
                    )
                ohv = oh.rearrange("p (pl c) -> p pl c", pl=2)
                e8v = e8.rearrange("p (nt dc m) -> p nt dc m", nt=KT, dc=DC)
                for dc in range(DC):
                    lhs = e8v[:, t0 : t0 + 2, dc, :]  # [P, 2, 128]
                    for ci, (c0, c1) in enumerate(CH):
                        nc.tensor.matmul(
                            s_ps[dc][ci][:],
                            lhs,
                            ohv[:, :, c0:c1],
                            start=(kp == 0), stop=(kp == KT // 2 - 1),
                            perf_mode=DR,
                        )
            # sums psum -> sbuf bf16 (D-major [512, 1000])
            sums_sb = cpool.tile([P, DC * C], BF16, name="sumssb")
            for dc in range(DC):
                for ci, (c0, c1) in enumerate(CH):
                    dsts = sums_sb[:, dc * C + c0 : dc * C + c1]
                    if (dc + ci) % 2 == 0:
                        nc.scalar.copy(dsts, s_ps[dc][ci][:])
                    else:
                        nc.vector.tensor_copy(out=dsts, in_=s_ps[dc][ci][:])

        sums_d = dram.tile([D, C], BF16, name="sumsd")
        for dc in range(DC):
            (nc.sync if dc % 2 == 0 else nc.gpsimd).dma_start(
                sums_d[dc * P : (dc + 1) * P, :],
                sums_sb[:, dc * C : (dc + 1) * C],
            )

        # ---- ReduceScatter: core i owns D rows [64i, 64i+64) ----
        rs_out = dram.tile([DS, C], BF16, name="rsout")
        nc.gpsimd.collective_compute(
            "ReduceScatter", ALU.add,
            replica_groups=[list(range(W))],
            ins=[sums_d.opt()], outs=[rs_out.opt()],
        )
        sums_rs = cpool.tile([P, C], BF16, name="sumsrs")
        nc.sync.dma_start(sums_rs[0:DS, :], rs_out[:])

        # ---- protos2T slice + psq partial, quantize, AllGather ----
        pr2 = cpool.tile([P, C], FP8, name="pr2")
        t2 = cpool.tile([P, C], F32, name="t2")
        sq8 = cpool.tile([P, C], FP8, name="sq8")
        for c0, c1 in CH:
            nc.vector.tensor_tensor(out=t2[0:DS, c0:c1],
                                    in0=sums_rs[0:DS, c0:c1],
                                    in1=bb_sb[0:DS, c0:c1], op=ALU.mult)
            nc.vector.tensor_tensor(out=pr2[0:DS, c0:c1],
                                    in0=t2[0:DS, c0:c1],
                                    in1=ap0_sb[0:DS, c0:c1], op=ALU.add)
            nc.vector.tensor_tensor(out=sq8[0:DS, c0:c1],
                                    in0=pr2[0:DS, c0:c1],
                                    in1=pr2[0:DS, c0:c1], op=ALU.mult)

        ag_in = dram.tile([AGB, C], FP8, name="agin")
        ag_out = dram.tile([W * AGB, C], FP8, name="agout",
                           addr_space="Shared")

        def _psq_ag():
            psq8 = cpool.tile([1, C], FP8, name="psq8")
            with tc.tile_pool(name="ps_pq", bufs=1, space="PSUM") as ps_pq:
                for ci, (c0, c1) in enumerate(CH):
                    pq = ps_pq.tile([1, c1 - c0], F32, tag=f"pq{ci}",
                                    name=f"pq{ci}")
                    nc.tensor.matmul(pq[:], ones8[0:DS, :], sq8[0:DS, c0:c1],
                                     start=True, stop=True)
                    nc.vector.tensor_scalar(psq8[0:1, c0:c1], pq[:],
                                            -0.25, None, ALU.mult)

            nc.sync.dma_start(ag_in[0:DS, :], pr2[0:DS, :])
            nc.sync.dma_start(ag_in[DS : DS + 1, :], psq8[:])
            nc.gpsimd.collective_compute(
                "AllGather", ALU.bypass,
                replica_groups=[list(range(W))],
                ins=[ag_in.opt()], outs=[ag_out.opt()],
            )

        # ---- emb transposes (fp8, stride-2 psum) fill the collective gap ----
        with tc.tile_pool(name="ps_tr", bufs=6, space="PSUM") as ps_tr:
            for t in range(KT):
                if t == 44:
                    _psq_ag()
                trb = ps_tr.tile([P, 2 * D], FP8, tag="trb", name="trb")
                trv = trb.rearrange("p (c two) -> p c two", two=2)
                for dc in range(DC):
                    nc.tensor.matmul(
                        trv[:, dc * P : (dc + 1) * P, 0:1],
                        e8[:, t * D + dc * P : t * D + (dc + 1) * P],
                        ident_8[:],
                        is_transpose=True,
                        start=(dc == 0), stop=(dc == DC - 1),
                    )
                dst8 = embT8[:, t * D : (t + 1) * D]
                nc.scalar.copy(dst8, trv[:, 0 : D, 0])


        # psq partial rows first (critical path to the fold rows)
        psqs = cpool.tile([8, C], FP8, name="psqs")
        nc.sync.dma_start(
            psqs[:],
            ag_out.rearrange("(k b) c -> k b c", b=AGB)[:, DS, :],
        )
        # protos2T blocks land pre-transposed: block k rows -> chunk layout
        agov = ag_out.rearrange("(dcq h b) c -> dcq h b c", dcq=DC, h=2)
        ptv = protosT8.rearrange("p (dcq c) -> p dcq c", dcq=DC)
        for h in range(2):
            (nc.scalar if h == 0 else nc.gpsimd).dma_start(
                ptv[h * DS : (h + 1) * DS, :, :],
                agov[:, h, 0:DS, :].rearrange("dcq b c -> b dcq c"),
            )
        with tc.tile_pool(name="ps_pf", bufs=1, space="PSUM") as ps_pf:
            for ci, (c0, c1) in enumerate(CH):
                pf = ps_pf.tile([1, c1 - c0], F32, tag=f"pf{ci}",
                                name=f"pf{ci}")
                nc.tensor.matmul(pf[:], ones8[0:8, :], psqs[:, c0:c1],
                                 start=True, stop=True)
                nc.vector.tensor_copy(out=fold_rhs[0:1, c0:c1],
                                      in_=pf[:])

        for et_d, t0 in deferred_esq:
            for i in range(2):
                scr = sq_pool.tile([P, D], BF16, tag="scr", name="scr")
                nc.scalar.activation(
                    scr[:], et_d[:, i * D : (i + 1) * D], ACTF.Square,
                    accum_out=esq_neg[:, t0 + i : t0 + i + 1],
                )

        # negate e_sq once (used as ScalarE bias in phase 2)
        nc.vector.tensor_scalar(esq_neg[:], esq_neg[:], -1.0, None, ALU.mult)

        # ================= phase 2 =================
        fones_v = fold_ones.rearrange("p (pl m) -> p pl m", pl=2)
        frhs_v = fold_rhs.rearrange("p (pl c) -> p pl c", pl=2)
        with tc.tile_pool(name="ps_cr", bufs=4, space="PSUM") as ps_cr:
            for nt in range(KT):
                ot = out_pool.tile([P, C], F16, tag="ot", name="ot")
                for ci, (c0, c1) in enumerate(CH):
                    cr = ps_cr.tile([P, c1 - c0], F32, tag=f"cr{ci}",
                                    name=f"cr{ci}")
                    nc.tensor.matmul(
                        cr[:], fones_v[:, :, :], frhs_v[:, :, c0:c1],
                        start=True, stop=False, perf_mode=DR,
                    )
                    for pr in range(2):
                        lhs = embT8[
                            :, nt * D + pr * 2 * P : nt * D + (pr + 1) * 2 * P
                        ].rearrange("p (pl m) -> p pl m", pl=2)
                        rhs = protosT8[
                            :, 2 * pr * C : (2 * pr + 2) * C
                        ].rearrange("p (pl c) -> p pl c", pl=2)[:, :, c0:c1]
                        nc.tensor.matmul(
                            cr[:], lhs, rhs,
                            start=False, stop=(pr == 1),
                            perf_mode=DR,
                        )
                    if (2 * nt + ci) % 2 == 0:
                        nc.scalar.activation(
                            ot[:, c0:c1], cr[:], ACTF.Identity,
                            bias=esq_neg[:, nt : nt + 1], scale=1.0,
                        )
                    else:
                        nc.vector.tensor_scalar(
                            ot[:, c0:c1], cr[:], esq_neg[:, nt : nt + 1],
                            None, ALU.add,
                        )
                st_eng = nc.sync if nt % 3 < 2 else nc.gpsimd
                st_eng.dma_start(out_ext[nt * P : (nt + 1) * P, :], ot[:])

    _split_waits(nc)
    return nc


def kernel(embeddings, prototypes, counter, y_true):
    embeddings = np.ascontiguousarray(np.asarray(embeddings, dtype=np.float32))
    prototypes = np.ascontiguousarray(np.asarray(prototypes, dtype=np.float32))
    counter_f = np.asarray(counter, dtype=np.float64)
    y = np.asarray(y_true).astype(np.int64)

    # host-side: counts + running-mean coefficients (index math only)
    counts = np.bincount(y, minlength=C).astype(np.float64)
    rep = counts > 0
    rm = 1.0 / np.maximum(counts, 1.0)
    rt = 1.0 / (counter_f + 1.0)
    B2 = (2.0 * rep * rm * rt).astype(np.float32)
    A2 = (2.0 * (1.0 + rep * (counter_f * rt - 1.0))).astype(np.float32)
    p0T = prototypes.T  # [D, C]

    if _built[0] is None:
        _built[0] = _build()
    nc = _built[0]

    in_maps = []
    for i in range(W):
        sl = slice(i * NL, (i + 1) * NL)
        ds = slice(i * DS, (i + 1) * DS)
        y_loc = y[sl].astype(np.float32)
        yf = np.ascontiguousarray(y_loc.reshape(KT, P).T)
        in_maps.append(
            {
                "emb": embeddings[sl],
                "yf": yf,
                "ap0": np.ascontiguousarray(A2[None, :] * p0T[ds]),
                "bb": np.ascontiguousarray(
                    np.broadcast_to(B2[None, :], (DS, C))
                ),
            }
        )

    res = run_bass_kernel_spmd(
        nc, in_maps, list(range(W)), trace=PROFILE, **TRACE_KWARGS
    )
    LAST_RESULT[0] = res
    out = np.concatenate([res.results[i]["out"] for i in range(W)], axis=0)
    return out.astype(np.float32)
